# revision 1
# baseline (speedup 1.0000x reference)
"""NF4-style 4-bit quantized linear: out = x @ dequant(w).T on 8 TRN2 NeuronCores.

Column-parallel sharding: core c owns output features [c*512, (c+1)*512) and the
corresponding contiguous slices of the packed weight + quant state arrays. x is
replicated. Each core:
  1. dequantizes its 512x4096 weight slice on-chip (DVE, fp16 output) into
     fp16, in 4 chunks of 8 k-tiles,
  2. round-trips each chunk through DRAM with an xbar transpose DMA to get wT
     [k-partition, outf] layout,
  3. streams x through xbar transpose DMAs ([token, k] -> [k, token]) and runs
     the fp16 matmul on the PE array, accumulating in PSUM over 32 k-tiles.
Scale-state + first packed-weight loads are issued before any transpose so the
dequant chain starts at ~10us; the xbar chain is pinned [xtr0, wt0, ...] so the
first matmul is gated only by the first x block + first W chunk. Steady x
blocks are transposed in 2MB halves for smoother supply. Output stores are
batched per 512-token group. Host gathers per-core outputs with a concat.
"""
import numpy as np

import concourse.bass as bass
import concourse.mybir as mybir
import concourse.tile as tile
from concourse import bacc
from concourse.tile_rust import add_dep_helper as tile_rust_add_dep
from concourse.bass_utils import run_bass_kernel_spmd

F16 = mybir.dt.float16
F32 = mybir.dt.float32
I32 = mybir.dt.int32
Alu = mybir.AluOpType

P = 128
TOKENS = 8192
IN_F = 4096
OUT_F = 4096
N_CORES = 8
O_C = OUT_F // N_CORES          # 512 out features per core
KT = IN_F // P                  # 32 k-tiles
BPR = IN_F // 2                 # 2048 packed bytes per weight row
NB_O = O_C // P                 # 4 o-tiles of 128 rows
TB = 512                        # steady token block (2 transpose halves)
HTB = TB // 2

NKC = 4                         # W chunks
KKC = KT // NKC                 # 8 k-tiles per chunk
KCW = IN_F // NKC               # 1024 k values per chunk
BCC = BPR // NKC                # 512 packed bytes per chunk (per row)
NBC = BCC // 32                 # 16 quant blocks per chunk (per row)

RTB = 256                       # ramp token block
NRB = 4                         # ramp blocks (4*256 = 1024 tokens = 8 psums)


def _build(tokens=TOKENS):
    nc = bacc.Bacc("TRN2", target_bir_lowering=False, debug=False,
                   enable_asserts=False)

    x = nc.dram_tensor("x", [tokens, IN_F], F16, kind="ExternalInput").ap()
    qw = nc.dram_tensor("qw", [O_C, BPR], I32, kind="ExternalInput").ap()
    qam = nc.dram_tensor("qam", [O_C, 64], I32, kind="ExternalInput").ap()
    qcode = nc.dram_tensor("qcode", [O_C, 64], F32, kind="ExternalInput").ap()
    qoff = nc.dram_tensor("qoff", [O_C, 64], F32, kind="ExternalInput").ap()
    am2 = nc.dram_tensor("am2", [O_C, 16], F32, kind="ExternalInput").ap()
    c2 = nc.dram_tensor("c2", [O_C, 16], F32, kind="ExternalInput").ap()
    out = nc.dram_tensor("out", [tokens, O_C], F16, kind="ExternalOutput").ap()

    n_steady = (tokens - NRB * RTB) // TB

    with tile.TileContext(nc) as tc:
        with tc.tile_pool(name="wt_pool", bufs=1) as wt_pool, \
             tc.tile_pool(name="wdram", bufs=1, space="DRAM") as wdram, \
             tc.tile_pool(name="sc_pool", bufs=1) as sc_pool, \
             tc.tile_pool(name="dq", bufs=2) as dq, \
             tc.tile_pool(name="xt_pool", bufs=2) as xt_pool, \
             tc.tile_pool(name="ps_pool", bufs=8, space="PSUM") as ps_pool, \
             tc.tile_pool(name="ob_pool", bufs=1) as ob_pool:

            # ---- gpsimd queue: qam (casting load) + batched chunk-0/1
            # packed-weight loads lead; everything dequant needs is in
            # flight before any xbar transpose is scheduled.
            am3 = sc_pool.tile([P, NB_O, 64], F32, name="am3")
            nc.gpsimd.dma_start(am3, qam.rearrange("(a p) c -> p a c", p=P))

            qw3 = qw.rearrange("(a p) c -> p a c", p=P)
            qts = {}

            def load_chunk(kc):
                qt = dq.tile([P, NB_O, BCC], I32, name="qt", bufs=2)
                nc.gpsimd.dma_start(
                    qt, qw3[:, :, kc * BCC:(kc + 1) * BCC])
                qts[kc] = qt

            load_chunk(0)

            # ---- remaining scale-state loads on the SP HWDGE ring
            cd3 = sc_pool.tile([P, NB_O, 64], F32, name="cd3")
            nc.sync.dma_start(cd3, qcode.rearrange("(a p) c -> p a c", p=P))
            c23 = sc_pool.tile([P, NB_O, 16], F32, name="c23")
            nc.sync.dma_start(c23, c2.rearrange("(a p) c -> p a c", p=P))
            am23 = sc_pool.tile([P, NB_O, 16], F32, name="am23")
            nc.sync.dma_start(am23, am2.rearrange("(a p) c -> p a c", p=P))
            of3 = sc_pool.tile([P, NB_O, 64], F32, name="of3")
            nc.sync.dma_start(of3, qoff.rearrange("(a p) c -> p a c", p=P))

            # ---- scale prep (DVE):  S = (am/code) * (am2/c2),  offS = off*S
            rc = sc_pool.tile([P, NB_O, 64], F32, name="rc")
            nc.vector.reciprocal(rc, cd3)
            s1 = sc_pool.tile([P, NB_O, 64], F32, name="s1")
            nc.vector.tensor_tensor(s1, am3, rc, Alu.mult)
            rc2 = sc_pool.tile([P, NB_O, 16], F32, name="rc2")
            nc.vector.reciprocal(rc2, c23)
            s2 = sc_pool.tile([P, NB_O, 16], F32, name="s2")
            nc.vector.tensor_tensor(s2, am23, rc2, Alu.mult)
            S3 = sc_pool.tile([P, NB_O, 64], F32, name="S3")
            nc.vector.tensor_tensor(
                S3, s1, s2.unsqueeze(3).broadcast_to([P, NB_O, 16, 4]), Alu.mult)
            offS3 = sc_pool.tile([P, NB_O, 64], F32, name="offS3")
            nc.vector.tensor_tensor(offS3, of3, S3, Alu.mult)

            # ---- dequant + W round-trip, chunk-major ----
            wts = []
            wt_insts = []
            for kc in range(NKC):
                if kc + 1 < NKC:
                    load_chunk(kc + 1)
                wd = wdram.tile([O_C, KCW], F16, name=f"wd{kc}")
                w_nat = dq.tile([P, NB_O, KCW], F16, name="wn", bufs=1)
                for ot in range(NB_O):
                    qt = qts[kc][:, ot, :]
                    hi = dq.tile([P, NBC, 32], I32, name="hi")
                    nc.vector.tensor_scalar(hi, qt, 4, None,
                                            Alu.logical_shift_right)
                    lo = dq.tile([P, NBC, 32], F16, name="lo")
                    nc.vector.scalar_tensor_tensor(
                        lo, hi, -16.0, qt, Alu.mult, Alu.add)
                    sb = S3[:, ot, kc * NBC:(kc + 1) * NBC] \
                        .unsqueeze(2).broadcast_to([P, NBC, 32])
                    mlo = dq.tile([P, NBC, 32], F16, name="mlo")
                    nc.vector.tensor_tensor(mlo, lo, sb, Alu.mult)
                    mhi = dq.tile([P, NBC, 32], F16, name="mhi")
                    nc.vector.tensor_tensor(mhi, hi, sb, Alu.mult)
                    offs = offS3[:, ot, kc * NBC:(kc + 1) * NBC] \
                        .unsqueeze(2).broadcast_to([P, NBC, 32])
                    nc.vector.tensor_tensor(w_nat[:, ot, 0::2], mlo, offs,
                                            Alu.subtract)
                    nc.vector.tensor_tensor(w_nat[:, ot, 1::2], mhi, offs,
                                            Alu.subtract)
                qts.pop(kc)
                for ot in range(NB_O):
                    rs = slice(ot * P, (ot + 1) * P)
                    nc.gpsimd.dma_start(wd[rs, :], w_nat[:, ot, :])
                wt = wt_pool.tile([P, KKC, O_C], F16, name=f"wt{kc}")
                wi = nc.scalar.dma_start(out=wt, in_=wd[:, :], transpose=True)
                wts.append(wt)
                wt_insts.append(wi)

            # ---- ramp x transposes ----
            xtr, xtr_insts = [], []
            for rb in range(NRB):
                t = xt_pool.tile([P, KT, RTB], F16, name=f"xtr{rb}", bufs=1)
                ti = nc.scalar.dma_start(
                    out=t, in_=x[rb * RTB:(rb + 1) * RTB, :], transpose=True)
                xtr.append(t)
                xtr_insts.append(ti)

            # ---- ramp matmuls: chunk-major across all ramp groups ----
            rps = [[ps_pool.tile([P, O_C], F32, name="ps")
                    for st in range(RTB // P)] for rb in range(NRB)]
            for kc in range(NKC):
                for rb in range(NRB):
                    for st in range(RTB // P):
                        for j in range(KKC):
                            kk = kc * KKC + j
                            nc.tensor.matmul(
                                rps[rb][st],
                                xtr[rb][:, kk, st * P:(st + 1) * P],
                                wts[kc][:, j, :],
                                start=(kk == 0),
                                stop=(kk == KT - 1),
                            )
            for g in range(NRB * RTB // TB):
                ob = ob_pool.tile([P, TB // P, O_C], F16, name="ob")
                for i in range(TB // P):
                    rb, st = divmod(g * (TB // P) + i, RTB // P)
                    nc.vector.tensor_copy(ob[:, i, :], rps[rb][st])
                r0 = g * TB
                nc.gpsimd.dma_start(
                    out[r0:r0 + TB, :].rearrange("(st p) c -> p st c", p=P),
                    ob)

            # ---- steady blocks ----
            base = NRB * RTB
            xt_insts = []
            for tb in range(n_steady):
                xt = xt_pool.tile([P, KT, TB], F16, name="xt")
                xi = nc.scalar.dma_start(
                    out=xt, in_=x[base + tb * TB: base + (tb + 1) * TB, :],
                    transpose=True)
                xt_insts.append(xi)
                ob = ob_pool.tile([P, TB // P, O_C], F16, name="ob")
                for st in range(TB // P):
                    ps = ps_pool.tile([P, O_C], F32, name="ps")
                    for kk in range(KT):
                        nc.tensor.matmul(
                            ps,
                            xt[:, kk, st * P:(st + 1) * P],
                            wts[kk // KKC][:, kk % KKC, :],
                            start=(kk == 0),
                            stop=(kk == KT - 1),
                        )
                    nc.vector.tensor_copy(ob[:, st, :], ps)
                r0 = base + tb * TB
                nc.gpsimd.dma_start(
                    out[r0:r0 + TB, :].rearrange("(st p) c -> p st c", p=P),
                    ob)

            # ---- pin the xbar ring order: first x block, then the first W
            # chunk (both gate the first matmul), then the rest interleaved
            # at the pace the ramp consumes them.
            chain = [xtr_insts[0], wt_insts[0], xtr_insts[1], xtr_insts[2],
                     wt_insts[1], xtr_insts[3], wt_insts[2], wt_insts[3]]
            chain += xt_insts
            for a, b in zip(chain[1:], chain):
                tile_rust_add_dep(a.ins, b.ins, True, "xbar order")

    nc.compile()
    return nc


_NC_CACHE = {}


def _get_nc(tokens=TOKENS):
    if tokens not in _NC_CACHE:
        _NC_CACHE[tokens] = _build(tokens)
    return _NC_CACHE[tokens]


def _shard(inputs):
    x = np.ascontiguousarray(np.asarray(inputs["x"], dtype=np.float16))
    qw = np.asarray(inputs["quantized_weight"], dtype=np.int32)
    qam = np.asarray(inputs["quant_absmax"], dtype=np.int32)
    qcode = np.asarray(inputs["quant_code"], dtype=np.float32)
    qoff = np.asarray(inputs["quant_offset"], dtype=np.float32)
    am2 = np.asarray(inputs["state2_absmax"], dtype=np.float32)
    c2 = np.asarray(inputs["state2_code"], dtype=np.float32)

    pb = O_C * BPR        # packed bytes per core
    nb1 = O_C * 64        # primary blocks per core
    nb2 = O_C * 16        # secondary blocks per core
    in_maps = []
    for c in range(N_CORES):
        in_maps.append({
            "x": x,
            "qw": np.ascontiguousarray(
                qw[c * pb:(c + 1) * pb].reshape(O_C, BPR)),
            "qam": np.ascontiguousarray(
                qam[c * nb1:(c + 1) * nb1].reshape(O_C, 64)),
            "qcode": np.ascontiguousarray(
                qcode[c * nb1:(c + 1) * nb1].reshape(O_C, 64)),
            "qoff": np.ascontiguousarray(
                qoff[c * nb1:(c + 1) * nb1].reshape(O_C, 64)),
            "am2": np.ascontiguousarray(
                am2[c * nb2:(c + 1) * nb2].reshape(O_C, 16)),
            "c2": np.ascontiguousarray(
                c2[c * nb2:(c + 1) * nb2].reshape(O_C, 16)),
        })
    return in_maps


def _run(inputs, trace=False, trace_cores=None):
    nc = _get_nc()
    in_maps = _shard(inputs)
    res = run_bass_kernel_spmd(
        nc, in_maps, list(range(N_CORES)), trace=trace,
        trace_cores=trace_cores)
    out = np.concatenate([r["out"] for r in res.results], axis=1)
    return out, res


def kernel(**inputs) -> np.ndarray:
    out, _ = _run(inputs, trace=False)
    return out

